# revision 81
# baseline (speedup 1.0000x reference)
"""Trainium2 Bass kernel for nn_AlignmentMatrix.

Math (per batch b):
    out[b,i,j] = ctx[b]@w1 [i] + asp[b]@w2 [j] + (ctx[b]*w3) @ asp[b].T [i,j]
with ctx [B,L1,H2]=[128,1024,600], asp [B,L2,H2]=[128,128,600],
w_u=[w1;w2;w3] each [600].

Device-side formulation (all FLOPs on device):
    rhs'[d,j]  = w3[d]*asp[b,j,d] + w1[d]        (DVE scale/bias, folds s_ctx;
                                                  batched over all b per chunk)
    s_asp[j]   = sum_d asp[b,j,d]*w2[d]          (PE, lhsT=asp chunk, rhs=w2
                                                  column -> PSUM column [L2,1])
    out_ps[j,i]= sum_d rhs'[d,j]*ctxT[d,i]       (PE, 5 K-chunks of 120)
    out[j,i]   = fp16(out_ps[j,i] + s_asp[j])    (DVE copy w/ per-partition
                                                  scalar add -> free rank-1)

The kernel is HBM-DMA-bound (16 shared DMA engines, ~18.8 GB/s each with
4KB descriptors, 15.2 GB/s with 5-6KB ones), so BYTES are time:
  * ctx AND asp ship as fp8-e3m4 (1-4-3): for N(0,1) data its 4-bit
    mantissa halves the quantization error vs e4m3 (host-sim rel err
    1.57e-2 vs the 2e-2 gate, vs 2.26e-2 all-e4m3).
  * ctx splits into ctxA (chunks 0-3, 4096B-aligned 4KB descriptors) and
    ctxB (chunk 4, two batches packed per row -> 2KB descriptors), both
    page-crossing-free.
  * reads AND writes spread greedily (byte-balanced) over all three
    dynamic DMA queues (sync/scalar HWDGE + gpsimd SWDGE).
A dummy-matmul warmup nudges the PE HAM toward K=8/8 before real work
lands (a cold PE runs matmuls at half rate and turns the tail into a
compute-paced drain).  fp32 accumulation in PSUM.

Sharding: data-parallel over batch, 16 batches per core across 8 cores.
"""

import numpy as np
import ml_dtypes

import concourse.bass as bass
import concourse.bacc as bacc
import concourse.mybir as mybir
import concourse.tile as tile
from concourse.bass_utils import run_bass_kernel_spmd

N_CORES = 8
B = 128
L1 = 1024  # ctx rows (i)
L2 = 128  # asp rows (j)
H = 600  # contraction dim (d)
BPC = B // N_CORES  # batches per core
KC = 5  # contraction chunks
KP = H // KC  # 120 rows per chunk
KA = 4  # leading chunks in ctxA (4KB descriptor rows)
NI = 512  # moving free-dim per matmul (one PSUM bank of fp32)
NIC = L1 // NI  # i-chunks per batch
OPACK = 2  # batches packed per output DMA
PF = 6  # ctx prefetch distance
NWARM = 6  # HAM ungates on cumulative PE busy; front-loaded sasp feeds it too
SASP_DVE = False  # s_asp on DVE measured ~1-2us SLOWER than 5 PE matmuls
SASP_EARLY = True  # bunch each part's s_asp PE matmuls early (batch is cross-only)

F32 = mybir.dt.float32
F16 = mybir.dt.float16
F8 = mybir.dt.float8e3
NP_F8 = ml_dtypes.float8_e3m4

CTXA_B = KP * KA * L1  # bytes of one batch of ctxA chunks (e3m4)
CTXB_B = KP * 2 * L1  # bytes of one PAIR of ctxB chunks (e3m4)
ASP_B = KP * KC * L2  # bytes of one batch of asp (e3m4)
OUT_B = L2 * OPACK * L1 * 2  # bytes of one output pair write

ASP_PARTS = [(0, 2), (2, 6), (6, 11), (11, BPC)]


def build_kernel():
    nc = bacc.Bacc(
        "TRN2", target_bir_lowering=False, debug=False, enable_asserts=False
    )
    ctxA = nc.dram_tensor(
        "ctxA", [BPC, KP, KA, L1], F8, kind="ExternalInput"
    ).ap()
    ctxB = nc.dram_tensor(
        "ctxB", [BPC // 2, KP, 2, L1], F8, kind="ExternalInput"
    ).ap()
    aspT = nc.dram_tensor(
        "aspT", [KP, BPC, KC, L2], F8, kind="ExternalInput"
    ).ap()
    wc = nc.dram_tensor("wc", [KP, 3 * KC], F32, kind="ExternalInput").ap()
    w2c = nc.dram_tensor("w2c", [KP, KC], F16, kind="ExternalInput").ap()
    outT = nc.dram_tensor(
        "outT", [BPC // OPACK, L2, OPACK, L1], F16, kind="ExternalOutput"
    ).ap()

    # All three dynamic DMA queues, greedy byte-balanced.
    dmae = [nc.sync, nc.scalar, nc.gpsimd]
    qbytes = [0, 0, 0]

    def q_issue(dst, src, nbytes, force=None):
        i = force if force is not None else min(range(3), key=lambda j: qbytes[j])
        qbytes[i] += nbytes
        dmae[i].dma_start(dst, src)

    MUL = mybir.AluOpType.mult
    ADD = mybir.AluOpType.add

    with tile.TileContext(nc) as tc:
        with (
            tc.tile_pool(name="consts", bufs=1) as consts,
            tc.tile_pool(name="cA_pool", bufs=PF + 1) as cA_pool,
            tc.tile_pool(name="cB_pool", bufs=PF // 2 + 2) as cB_pool,
            tc.tile_pool(name="asp_pool", bufs=1) as asp_pool,
            tc.tile_pool(name="rhsp_pool", bufs=1) as rhsp_pool,
            tc.tile_pool(name="sasp_pool", bufs=1) as sasp_pool,
            tc.tile_pool(name="out_pool", bufs=3) as out_pool,
            tc.tile_pool(
                name="ps_out",
                bufs=6 if (SASP_DVE or SASP_EARLY) else 7,
                space="PSUM",
            ) as ps_out,
            tc.tile_pool(
                name="ps_sasp",
                bufs=2 if (SASP_DVE or SASP_EARLY) else 1,
                space="PSUM",
            ) as ps_sasp,
            tc.tile_pool(name="u_pool", bufs=2) as u_pool,
        ):
            wc_t = consts.tile([KP, 3 * KC], F32)
            w2c_t = consts.tile([KP, KC], F16)
            asp_t = asp_pool.tile([KP, BPC, KC, L2], F8)
            rhsp_t = rhsp_pool.tile([KP, BPC, KC, L2], F16)
            sasp_t = sasp_pool.tile([L2, BPC], F32)
            dummy = consts.tile([KP, NI], F16)
            ones_t = consts.tile([KP, 1], F16)

            ctxA_tiles = {}
            ctxB_tiles = {}

            def load_ctx(b, qa=None, qb=None):
                tA = cA_pool.tile([KP, KA, L1], F8, tag="cA")
                q_issue(tA[:], ctxA[b], CTXA_B, force=qa)
                ctxA_tiles[b] = tA
                if b % 2 == 0:
                    tB = cB_pool.tile([KP, 2, L1], F8, tag="cB")
                    q_issue(tB[:], ctxB[b // 2], CTXB_B, force=qb)
                    ctxB_tiles[b // 2] = tB

            def load_asp(g, force=None):
                lo, hi = ASP_PARTS[g]
                q_issue(
                    asp_t[:, lo:hi], aspT[:, lo:hi], (hi - lo) * ASP_B, force=force
                )

            def rhsp_round(g):
                # rhs'[d,b,k,j] = w3[d,k]*asp[d,b,k,j] + w1[d,k], one DVE op
                # per chunk k batched over the part's batches.
                lo, hi = ASP_PARTS[g]
                for k in range(KC):
                    nc.vector.tensor_scalar(
                        rhsp_t[:, lo:hi, k, :],
                        asp_t[:, lo:hi, k, :],
                        wc_t[:, KC + k : KC + k + 1],
                        wc_t[:, k : k + 1],
                        MUL,
                        ADD,
                    )

            def sasp_group(b):
                # s_asp as a PSUM column: out[j,1] = sum_d asp[d,j]*w2[d]
                sasp_ps = ps_sasp.tile([L2, 1], F32, tag="sasp")
                for k in range(KC):
                    nc.tensor.matmul(
                        sasp_ps[:],
                        asp_t[:, b, k, :],
                        w2c_t[:, k : k + 1],
                        start=(k == 0),
                        stop=(k == KC - 1),
                    )
                nc.vector.tensor_copy(sasp_t[:, b : b + 1], sasp_ps[:])

            def sasp_round(g):
                # s_asp via a bulk DVE FMA chain over the whole asp part:
                #   u[d,b,j] = sum_k w2[d,k]*asp[d,b,k,j]
                # then ONE 1-col PE matmul per batch (vs 5):
                #   sasp_ps[j,1] = sum_d u[d,b,j]*1
                # Bunched at part boundaries so the PE stream never waits
                # on the DVE mid-batch.
                lo, hi = ASP_PARTS[g]
                u = u_pool.tile([KP, 5, L2], F16, tag="u")
                nc.vector.tensor_scalar(
                    u[:, : hi - lo, :],
                    asp_t[:, lo:hi, 0, :],
                    wc_t[:, 2 * KC : 2 * KC + 1],
                    None,
                    MUL,
                )
                for k in range(1, KC):
                    nc.vector.scalar_tensor_tensor(
                        u[:, : hi - lo, :],
                        asp_t[:, lo:hi, k, :],
                        wc_t[:, 2 * KC + k : 2 * KC + k + 1],
                        u[:, : hi - lo, :],
                        MUL,
                        ADD,
                    )
                for b in range(lo, hi):
                    sasp_ps = ps_sasp.tile([L2, 1], F32, tag="sasp")
                    nc.tensor.matmul(sasp_ps[:], u[:, b - lo, :], ones_t[:])
                    nc.vector.tensor_copy(sasp_t[:, b : b + 1], sasp_ps[:])

            # Startup: the three big first reads hit three distinct queues
            # immediately; the tiny const loads ride behind them.
            nc.gpsimd.memset(dummy[:], 0.0)
            nc.gpsimd.memset(ones_t[:], 1.0)
            # Split the first loads across all three queues so descriptor
            # generation runs in parallel: the engine-limited read stream
            # starts ~0.7us earlier, shifting the whole pipeline left.
            tA0 = cA_pool.tile([KP, KA, L1], F8, tag="cA")
            nc.sync.dma_start(tA0[:, 0:2], ctxA[0, :, 0:2])
            nc.scalar.dma_start(tA0[:, 2:4], ctxA[0, :, 2:4])
            qbytes[0] += CTXA_B // 2
            qbytes[1] += CTXA_B // 2
            ctxA_tiles[0] = tA0
            tB0 = cB_pool.tile([KP, 2, L1], F8, tag="cB")
            nc.gpsimd.dma_start(tB0[:], ctxB[0])
            qbytes[2] += CTXB_B
            ctxB_tiles[0] = tB0
            load_asp(0, force=0)
            nc.gpsimd.dma_start(w2c_t[:], w2c[:])
            nc.gpsimd.dma_start(wc_t[:], wc[:])

            # PE warmup on dummy data: HAM un-throttles after sustained
            # activity; real work feeds the integrator too, so default off.
            if NWARM:
                warm_ps = ps_out.tile([L2, NI], F32, tag="out_ps")
                for _ in range(NWARM):
                    nc.tensor.matmul(warm_ps[:], dummy[:, 0:L2], dummy[:])

            rhsp_round(0)
            if SASP_DVE:
                sasp_round(0)
            elif SASP_EARLY:
                for bb in range(*ASP_PARTS[0]):
                    sasp_group(bb)
            load_asp(1)
            for b in range(1, PF):
                load_ctx(b)
            load_asp(2)
            load_asp(3)

            out_sb = None
            pend_writes = []
            for b in range(BPC):
                if b + PF < BPC:
                    load_ctx(b + PF)
                tA = ctxA_tiles.pop(b)
                tB = ctxB_tiles[b // 2] if b % 2 == 0 else ctxB_tiles.pop(b // 2)

                if not (SASP_DVE or SASP_EARLY):
                    sasp_group(b)

                if b % OPACK == 0:
                    out_sb = out_pool.tile([L2, OPACK, L1], F16, tag="out")
                # k-major: both i-chunks accumulate in parallel PSUM groups,
                # so each stationary rhs' chunk is loaded ONCE per batch
                # (halves LDWEIGHTS) and consecutive matmuls share it.
                out_ps = [
                    ps_out.tile([L2, NI], F32, tag="out_ps", name=f"ops_{b}_{c}")
                    for c in range(NIC)
                ]
                for k in range(KC):
                    for c in range(NIC):
                        rhs = (
                            tA[:, k, c * NI : (c + 1) * NI]
                            if k < KA
                            else tB[:, b % 2, c * NI : (c + 1) * NI]
                        )
                        nc.tensor.matmul(
                            out_ps[c][:],
                            rhsp_t[:, b, k, :],
                            rhs,
                            start=(k == 0),
                            stop=(k == KC - 1),
                        )
                for c in range(NIC):
                    # fp16(out_ps + s_asp[j]) -> SBUF; the rank-1 term rides
                    # the PSUM-drain copy as a per-partition scalar add.
                    nc.vector.tensor_scalar(
                        out_sb[:, b % OPACK, c * NI : (c + 1) * NI],
                        out_ps[c][:],
                        sasp_t[:, b : b + 1],
                        None,
                        ADD,
                    )

                # Interleave the remaining rhsp rounds behind early batches
                # so each round's asp part has landed by the time DVE reaches
                # it, without blocking batch-0/1 output copies.
                if b < len(ASP_PARTS) - 1:
                    rhsp_round(b + 1)
                    if SASP_DVE:
                        sasp_round(b + 1)
                    elif SASP_EARLY:
                        for bb in range(*ASP_PARTS[b + 1]):
                            sasp_group(bb)

                if b >= BPC - 2 * OPACK:
                    # tail: flush the delayed pair write, then per-batch
                    # half-pair writes on parallel queues
                    while pend_writes:
                        dst, src_t = pend_writes.pop(0)
                        q_issue(dst, src_t[:], OUT_B)
                    q_issue(
                        outT[b // OPACK, :, b % OPACK : b % OPACK + 1, :],
                        out_sb[:, b % OPACK : b % OPACK + 1, :],
                        OUT_B // 2,
                    )
                elif b % OPACK == OPACK - 1:
                    # delay the pair write by one pair so its data is already
                    # computed when the DMA instruction reaches the head of
                    # its queue (a waiting write blocks the reads behind it)
                    pend_writes.append((outT[b // OPACK], out_sb))
                    if len(pend_writes) > 1:
                        dst, src_t = pend_writes.pop(0)
                        q_issue(dst, src_t[:], OUT_B)

    nc.compile()
    return nc


_NC_CACHE = None


def _get_nc():
    global _NC_CACHE
    if _NC_CACHE is None:
        _NC_CACHE = build_kernel()
    return _NC_CACHE


def kernel(batch_size=None, ctx=None, asp=None, w_u=None, **run_kwargs):
    ctx = np.asarray(ctx, dtype=np.float32)
    asp = np.asarray(asp, dtype=np.float32)
    w_u = np.asarray(w_u, dtype=np.float32).reshape(-1)
    w1, w2, w3 = w_u[:H], w_u[H : 2 * H], w_u[2 * H :]

    # Host-side layout transforms + dtype cast (partition-major so every
    # DMA descriptor is a long page-crossing-free contiguous run).
    # cT[b, p, k, i] = ctx[b, i, k*KP+p]
    cT = ctx.reshape(B, L1, KC, KP).transpose(0, 3, 2, 1)  # [B, KP, KC, L1]
    cT8 = cT.astype(NP_F8)
    ctxA_h = np.ascontiguousarray(cT8[:, :, :KA])  # [B, KP, KA, L1]
    # ctxB packs chunk KA of batch pairs: [B//2, KP, 2, L1]
    cB = cT8[:, :, KA]  # [B, KP, L1]
    ctxB_h = np.ascontiguousarray(
        cB.reshape(B // 2, 2, KP, L1).transpose(0, 2, 1, 3)
    )
    # aspT[p, b, k, j] = asp[b, j, k*KP+p]  (b local per core at slice time)
    aT = asp.reshape(B, L2, KC, KP).transpose(3, 0, 2, 1)  # [KP, B, KC, L2]
    aspT_h = np.ascontiguousarray(aT).astype(NP_F8)
    # wc[p, 2*KC]: w1 chunk-cols | w3 (fp32, DVE scale/bias); w2c separate.
    wc = np.ascontiguousarray(
        np.concatenate(
            [w1.reshape(KC, KP).T, w3.reshape(KC, KP).T, w2.reshape(KC, KP).T],
            axis=1,
        )
    ).astype(np.float32)
    w2c = np.ascontiguousarray(w2.reshape(KC, KP).T).astype(np.float16)

    nc = _get_nc()
    in_maps = [
        {
            "ctxA": ctxA_h[c * BPC : (c + 1) * BPC],
            "ctxB": ctxB_h[c * (BPC // 2) : (c + 1) * (BPC // 2)],
            "aspT": np.ascontiguousarray(aspT_h[:, c * BPC : (c + 1) * BPC]),
            "wc": wc,
            "w2c": w2c,
        }
        for c in range(N_CORES)
    ]
    res = run_bass_kernel_spmd(
        nc, in_maps, core_ids=list(range(N_CORES)), **run_kwargs
    )
    outT = np.concatenate(
        [res.results[c]["outT"] for c in range(N_CORES)], axis=0
    ).astype(np.float32)  # [B//OPACK, L2, OPACK, L1]
    out = np.ascontiguousarray(
        outT.transpose(0, 2, 3, 1).reshape(B, L1, L2)
    )  # [B, L1, L2]
    if run_kwargs:
        return out, res
    return out


# revision 83
# speedup vs baseline: 1.0148x; 1.0148x over previous
"""Trainium2 Bass kernel for nn_AlignmentMatrix.

Math (per batch b):
    out[b,i,j] = ctx[b]@w1 [i] + asp[b]@w2 [j] + (ctx[b]*w3) @ asp[b].T [i,j]
with ctx [B,L1,H2]=[128,1024,600], asp [B,L2,H2]=[128,128,600],
w_u=[w1;w2;w3] each [600].

Device-side formulation (all FLOPs on device):
    rhs'[d,j]  = w3[d]*asp[b,j,d] + w1[d]        (DVE scale/bias, folds s_ctx;
                                                  batched over all b per chunk)
    s_asp[j]   = sum_d asp[b,j,d]*w2[d]          (PE, lhsT=asp chunk, rhs=w2
                                                  column -> PSUM column [L2,1])
    out_ps[j,i]= sum_d rhs'[d,j]*ctxT[d,i]       (PE, 5 K-chunks of 120)
    out[j,i]   = fp16(out_ps[j,i] + s_asp[j])    (DVE copy w/ per-partition
                                                  scalar add -> free rank-1)

The kernel is HBM-DMA-bound (16 shared DMA engines, ~18.8 GB/s each with
4KB descriptors, 15.2 GB/s with 5-6KB ones), so BYTES are time:
  * ctx AND asp ship as fp8-e3m4 (1-4-3): for N(0,1) data its 4-bit
    mantissa halves the quantization error vs e4m3 (host-sim rel err
    1.57e-2 vs the 2e-2 gate, vs 2.26e-2 all-e4m3).
  * ctx splits into ctxA (chunks 0-3, 4096B-aligned 4KB descriptors) and
    ctxB (chunk 4, two batches packed per row -> 2KB descriptors), both
    page-crossing-free.
  * reads AND writes spread greedily (byte-balanced) over all three
    dynamic DMA queues (sync/scalar HWDGE + gpsimd SWDGE).
A dummy-matmul warmup nudges the PE HAM toward K=8/8 before real work
lands (a cold PE runs matmuls at half rate and turns the tail into a
compute-paced drain).  fp32 accumulation in PSUM.

Sharding: data-parallel over batch, 16 batches per core across 8 cores.
"""

import numpy as np
import ml_dtypes

import concourse.bass as bass
import concourse.bacc as bacc
import concourse.mybir as mybir
import concourse.tile as tile
from concourse.bass_utils import run_bass_kernel_spmd

N_CORES = 8
B = 128
L1 = 1024  # ctx rows (i)
L2 = 128  # asp rows (j)
H = 600  # contraction dim (d)
BPC = B // N_CORES  # batches per core
KC = 5  # contraction chunks
KP = H // KC  # 120 rows per chunk
KA = 4  # leading chunks in ctxA (4KB descriptor rows)
NI = 512  # moving free-dim per matmul (one PSUM bank of fp32)
NIC = L1 // NI  # i-chunks per batch
OPACK = 2  # batches packed per output DMA
PF = 6  # ctx prefetch distance
NWARM = 10  # HAM ungates on cumulative PE busy; front-loaded sasp feeds it too
SASP_DVE = False  # s_asp on DVE measured ~1-2us SLOWER than 5 PE matmuls
SASP_EARLY = True  # bunch each part's s_asp PE matmuls early (batch is cross-only)

F32 = mybir.dt.float32
F16 = mybir.dt.float16
F8 = mybir.dt.float8e3
NP_F8 = ml_dtypes.float8_e3m4

CTXA_B = KP * KA * L1  # bytes of one batch of ctxA chunks (e3m4)
CTXB_B = KP * 2 * L1  # bytes of one PAIR of ctxB chunks (e3m4)
ASP_B = KP * KC * L2  # bytes of one batch of asp (e3m4)
OUT_B = L2 * OPACK * L1 * 2  # bytes of one output pair write

ASP_PARTS = [(0, 2), (2, 6), (6, 11), (11, BPC)]


def build_kernel():
    nc = bacc.Bacc(
        "TRN2", target_bir_lowering=False, debug=False, enable_asserts=False
    )
    ctxA = nc.dram_tensor(
        "ctxA", [BPC, KP, KA, L1], F8, kind="ExternalInput"
    ).ap()
    ctxB = nc.dram_tensor(
        "ctxB", [BPC // 2, KP, 2, L1], F8, kind="ExternalInput"
    ).ap()
    aspT = nc.dram_tensor(
        "aspT", [KP, BPC, KC, L2], F8, kind="ExternalInput"
    ).ap()
    wc = nc.dram_tensor("wc", [KP, 3 * KC], F32, kind="ExternalInput").ap()
    w2c = nc.dram_tensor("w2c", [KP, KC], F16, kind="ExternalInput").ap()
    outT = nc.dram_tensor(
        "outT", [BPC // OPACK, L2, OPACK, L1], F16, kind="ExternalOutput"
    ).ap()

    # All three dynamic DMA queues, greedy byte-balanced.
    dmae = [nc.sync, nc.scalar, nc.gpsimd]
    qbytes = [0, 0, 0]

    def q_issue(dst, src, nbytes, force=None):
        i = force if force is not None else min(range(3), key=lambda j: qbytes[j])
        qbytes[i] += nbytes
        dmae[i].dma_start(dst, src)

    MUL = mybir.AluOpType.mult
    ADD = mybir.AluOpType.add

    with tile.TileContext(nc) as tc:
        with (
            tc.tile_pool(name="consts", bufs=1) as consts,
            tc.tile_pool(name="cA_pool", bufs=PF + 1) as cA_pool,
            tc.tile_pool(name="cB_pool", bufs=PF // 2 + 2) as cB_pool,
            tc.tile_pool(name="asp_pool", bufs=1) as asp_pool,
            tc.tile_pool(name="rhsp_pool", bufs=1) as rhsp_pool,
            tc.tile_pool(name="sasp_pool", bufs=1) as sasp_pool,
            tc.tile_pool(name="out_pool", bufs=3) as out_pool,
            tc.tile_pool(
                name="ps_out",
                bufs=6 if (SASP_DVE or SASP_EARLY) else 7,
                space="PSUM",
            ) as ps_out,
            tc.tile_pool(
                name="ps_sasp",
                bufs=2 if (SASP_DVE or SASP_EARLY) else 1,
                space="PSUM",
            ) as ps_sasp,
            tc.tile_pool(name="u_pool", bufs=2) as u_pool,
        ):
            wc_t = consts.tile([KP, 3 * KC], F32)
            w2c_t = consts.tile([KP, KC], F16)
            asp_t = asp_pool.tile([KP, BPC, KC, L2], F8)
            rhsp_t = rhsp_pool.tile([KP, BPC, KC, L2], F16)
            sasp_t = sasp_pool.tile([L2, BPC], F32)
            dummy = consts.tile([KP, NI], F16)
            ones_t = consts.tile([KP, 1], F16)

            ctxA_tiles = {}
            ctxB_tiles = {}

            def load_ctx(b, qa=None, qb=None):
                tA = cA_pool.tile([KP, KA, L1], F8, tag="cA")
                q_issue(tA[:], ctxA[b], CTXA_B, force=qa)
                ctxA_tiles[b] = tA
                if b % 2 == 0:
                    tB = cB_pool.tile([KP, 2, L1], F8, tag="cB")
                    q_issue(tB[:], ctxB[b // 2], CTXB_B, force=qb)
                    ctxB_tiles[b // 2] = tB

            def load_asp(g, force=None):
                lo, hi = ASP_PARTS[g]
                q_issue(
                    asp_t[:, lo:hi], aspT[:, lo:hi], (hi - lo) * ASP_B, force=force
                )

            def rhsp_round(g):
                # rhs'[d,b,k,j] = w3[d,k]*asp[d,b,k,j] + w1[d,k], one DVE op
                # per chunk k batched over the part's batches.
                lo, hi = ASP_PARTS[g]
                for k in range(KC):
                    nc.vector.tensor_scalar(
                        rhsp_t[:, lo:hi, k, :],
                        asp_t[:, lo:hi, k, :],
                        wc_t[:, KC + k : KC + k + 1],
                        wc_t[:, k : k + 1],
                        MUL,
                        ADD,
                    )

            def sasp_group(b):
                # s_asp as a PSUM column: out[j,1] = sum_d asp[d,j]*w2[d]
                sasp_ps = ps_sasp.tile([L2, 1], F32, tag="sasp")
                for k in range(KC):
                    nc.tensor.matmul(
                        sasp_ps[:],
                        asp_t[:, b, k, :],
                        w2c_t[:, k : k + 1],
                        start=(k == 0),
                        stop=(k == KC - 1),
                    )
                nc.vector.tensor_copy(sasp_t[:, b : b + 1], sasp_ps[:])

            def sasp_round(g):
                # s_asp via a bulk DVE FMA chain over the whole asp part:
                #   u[d,b,j] = sum_k w2[d,k]*asp[d,b,k,j]
                # then ONE 1-col PE matmul per batch (vs 5):
                #   sasp_ps[j,1] = sum_d u[d,b,j]*1
                # Bunched at part boundaries so the PE stream never waits
                # on the DVE mid-batch.
                lo, hi = ASP_PARTS[g]
                u = u_pool.tile([KP, 5, L2], F16, tag="u")
                nc.vector.tensor_scalar(
                    u[:, : hi - lo, :],
                    asp_t[:, lo:hi, 0, :],
                    wc_t[:, 2 * KC : 2 * KC + 1],
                    None,
                    MUL,
                )
                for k in range(1, KC):
                    nc.vector.scalar_tensor_tensor(
                        u[:, : hi - lo, :],
                        asp_t[:, lo:hi, k, :],
                        wc_t[:, 2 * KC + k : 2 * KC + k + 1],
                        u[:, : hi - lo, :],
                        MUL,
                        ADD,
                    )
                for b in range(lo, hi):
                    sasp_ps = ps_sasp.tile([L2, 1], F32, tag="sasp")
                    nc.tensor.matmul(sasp_ps[:], u[:, b - lo, :], ones_t[:])
                    nc.vector.tensor_copy(sasp_t[:, b : b + 1], sasp_ps[:])

            # Startup: the three big first reads hit three distinct queues
            # immediately; the tiny const loads ride behind them.
            nc.gpsimd.memset(dummy[:], 0.0)
            nc.gpsimd.memset(ones_t[:], 1.0)
            # Split the first loads across all three queues so descriptor
            # generation runs in parallel: the engine-limited read stream
            # starts ~0.7us earlier, shifting the whole pipeline left.
            tA0 = cA_pool.tile([KP, KA, L1], F8, tag="cA")
            nc.sync.dma_start(tA0[:, 0:2], ctxA[0, :, 0:2])
            nc.scalar.dma_start(tA0[:, 2:4], ctxA[0, :, 2:4])
            qbytes[0] += CTXA_B // 2
            qbytes[1] += CTXA_B // 2
            ctxA_tiles[0] = tA0
            tB0 = cB_pool.tile([KP, 2, L1], F8, tag="cB")
            nc.gpsimd.dma_start(tB0[:], ctxB[0])
            qbytes[2] += CTXB_B
            ctxB_tiles[0] = tB0
            load_asp(0, force=0)
            nc.gpsimd.dma_start(w2c_t[:], w2c[:])
            nc.gpsimd.dma_start(wc_t[:], wc[:])

            # PE warmup on dummy data: HAM un-throttles after sustained
            # activity; real work feeds the integrator too, so default off.
            if NWARM:
                warm_ps = ps_out.tile([L2, NI], F32, tag="out_ps")
                for _ in range(NWARM):
                    nc.tensor.matmul(warm_ps[:], dummy[:, 0:L2], dummy[:])

            rhsp_round(0)
            if SASP_DVE:
                sasp_round(0)
            elif SASP_EARLY:
                for bb in range(*ASP_PARTS[0]):
                    sasp_group(bb)
            load_asp(1)
            for b in range(1, PF):
                load_ctx(b)
            load_asp(2)
            load_asp(3)

            out_sb = None
            pend_writes = []
            for b in range(BPC):
                if b + PF < BPC:
                    load_ctx(b + PF)
                tA = ctxA_tiles.pop(b)
                tB = ctxB_tiles[b // 2] if b % 2 == 0 else ctxB_tiles.pop(b // 2)

                if not (SASP_DVE or SASP_EARLY):
                    sasp_group(b)

                if b % OPACK == 0:
                    out_sb = out_pool.tile([L2, OPACK, L1], F16, tag="out")
                # k-major: both i-chunks accumulate in parallel PSUM groups,
                # so each stationary rhs' chunk is loaded ONCE per batch
                # (halves LDWEIGHTS) and consecutive matmuls share it.
                out_ps = [
                    ps_out.tile([L2, NI], F32, tag="out_ps", name=f"ops_{b}_{c}")
                    for c in range(NIC)
                ]
                for k in range(KC):
                    for c in range(NIC):
                        rhs = (
                            tA[:, k, c * NI : (c + 1) * NI]
                            if k < KA
                            else tB[:, b % 2, c * NI : (c + 1) * NI]
                        )
                        nc.tensor.matmul(
                            out_ps[c][:],
                            rhsp_t[:, b, k, :],
                            rhs,
                            start=(k == 0),
                            stop=(k == KC - 1),
                        )
                for c in range(NIC):
                    # fp16(out_ps + s_asp[j]) -> SBUF; the rank-1 term rides
                    # the PSUM-drain copy as a per-partition scalar add.
                    nc.vector.tensor_scalar(
                        out_sb[:, b % OPACK, c * NI : (c + 1) * NI],
                        out_ps[c][:],
                        sasp_t[:, b : b + 1],
                        None,
                        ADD,
                    )

                # Interleave the remaining rhsp rounds behind early batches
                # so each round's asp part has landed by the time DVE reaches
                # it, without blocking batch-0/1 output copies.
                if b < len(ASP_PARTS) - 1:
                    rhsp_round(b + 1)
                    if SASP_DVE:
                        sasp_round(b + 1)
                    elif SASP_EARLY:
                        for bb in range(*ASP_PARTS[b + 1]):
                            sasp_group(bb)

                if b >= BPC - 2 * OPACK:
                    # tail: flush the delayed pair write, then per-batch
                    # half-pair writes on parallel queues
                    while pend_writes:
                        dst, src_t = pend_writes.pop(0)
                        q_issue(dst, src_t[:], OUT_B)
                    q_issue(
                        outT[b // OPACK, :, b % OPACK : b % OPACK + 1, :],
                        out_sb[:, b % OPACK : b % OPACK + 1, :],
                        OUT_B // 2,
                    )
                elif b % OPACK == OPACK - 1:
                    # delay the pair write by one pair so its data is already
                    # computed when the DMA instruction reaches the head of
                    # its queue (a waiting write blocks the reads behind it)
                    pend_writes.append((outT[b // OPACK], out_sb))
                    if len(pend_writes) > 1:
                        dst, src_t = pend_writes.pop(0)
                        q_issue(dst, src_t[:], OUT_B)

    nc.compile()
    return nc


_NC_CACHE = None


def _get_nc():
    global _NC_CACHE
    if _NC_CACHE is None:
        _NC_CACHE = build_kernel()
    return _NC_CACHE


def kernel(batch_size=None, ctx=None, asp=None, w_u=None, **run_kwargs):
    ctx = np.asarray(ctx, dtype=np.float32)
    asp = np.asarray(asp, dtype=np.float32)
    w_u = np.asarray(w_u, dtype=np.float32).reshape(-1)
    w1, w2, w3 = w_u[:H], w_u[H : 2 * H], w_u[2 * H :]

    # Host-side layout transforms + dtype cast (partition-major so every
    # DMA descriptor is a long page-crossing-free contiguous run).
    # cT[b, p, k, i] = ctx[b, i, k*KP+p]
    cT = ctx.reshape(B, L1, KC, KP).transpose(0, 3, 2, 1)  # [B, KP, KC, L1]
    cT8 = cT.astype(NP_F8)
    ctxA_h = np.ascontiguousarray(cT8[:, :, :KA])  # [B, KP, KA, L1]
    # ctxB packs chunk KA of batch pairs: [B//2, KP, 2, L1]
    cB = cT8[:, :, KA]  # [B, KP, L1]
    ctxB_h = np.ascontiguousarray(
        cB.reshape(B // 2, 2, KP, L1).transpose(0, 2, 1, 3)
    )
    # aspT[p, b, k, j] = asp[b, j, k*KP+p]  (b local per core at slice time)
    aT = asp.reshape(B, L2, KC, KP).transpose(3, 0, 2, 1)  # [KP, B, KC, L2]
    aspT_h = np.ascontiguousarray(aT).astype(NP_F8)
    # wc[p, 2*KC]: w1 chunk-cols | w3 (fp32, DVE scale/bias); w2c separate.
    wc = np.ascontiguousarray(
        np.concatenate(
            [w1.reshape(KC, KP).T, w3.reshape(KC, KP).T, w2.reshape(KC, KP).T],
            axis=1,
        )
    ).astype(np.float32)
    w2c = np.ascontiguousarray(w2.reshape(KC, KP).T).astype(np.float16)

    nc = _get_nc()
    in_maps = [
        {
            "ctxA": ctxA_h[c * BPC : (c + 1) * BPC],
            "ctxB": ctxB_h[c * (BPC // 2) : (c + 1) * (BPC // 2)],
            "aspT": np.ascontiguousarray(aspT_h[:, c * BPC : (c + 1) * BPC]),
            "wc": wc,
            "w2c": w2c,
        }
        for c in range(N_CORES)
    ]
    res = run_bass_kernel_spmd(
        nc, in_maps, core_ids=list(range(N_CORES)), **run_kwargs
    )
    outT = np.concatenate(
        [res.results[c]["outT"] for c in range(N_CORES)], axis=0
    ).astype(np.float32)  # [B//OPACK, L2, OPACK, L1]
    out = np.ascontiguousarray(
        outT.transpose(0, 2, 3, 1).reshape(B, L1, L2)
    )  # [B, L1, L2]
    if run_kwargs:
        return out, res
    return out
